# revision 6
# baseline (speedup 1.0000x reference)
"""Bass/Tile TRN2 kernel for nn_AverageAttention (cumavg -> LN -> FFN -> sigmoid gating).

Sharding: data-parallel over batch, one batch element per NeuronCore (B=8, 8 cores).

Per-core pipeline (L=2048 tokens processed in 4 quarters of 512 tokens = 4 tiles
of 128):
  phase A (per 128-token tile, natural [t, d] layout):
     cumavg via triu-matmul + running-prefix (strict-lower-tril matmul) in a
     persistent PSUM region; LayerNorm stats via bn_stats/bn_aggr; PE-transposes
     produce xT / avgT(+b2) / lnT chunks in [d, t] layout.
  phase B (per quarter): y1T = w1'@lnT (relu, +b1'), y2T = w2@r1T, ffnT = y2T + avgT;
     ffn also transposed back to natural and written out.
  phase C (per quarter): gating gT = gw@[xT; ffnT] streamed per 128-col slice of gw,
     sigmoid (+gb), outT = sig_ig*xT + sig_fg*ffnT, transposed back and written out.

ln_g/ln_b are folded into w1/b1 on the host (LN(x)*g+b @ w1 + b1 ==
LNhat(x) @ (g*w1) + (b@w1+b1)).  All matmuls run as float32r (full-rate fp32);
tensors feeding a matmul are declared float32r end-to-end (DMA'd straight from
f32r DRAM, or written with an f32->f32r cast by DVE) to satisfy the BIR
verifier's rounding rule.
"""

import numpy as np

B, L, D = 8, 2048, 1024
P = 128
NT = L // P          # 16 token tiles
KC = D // P          # 8 d-chunks
GC = 2 * D // P      # 16 gating chunks
QT = 4               # tiles per quarter
NQ = NT // QT        # 4 quarters
QW = QT * P          # 512 tokens per quarter
EPS = 1e-6

_CACHE = {}


def _build():
    if "nc" in _CACHE:
        return _CACHE["nc"]

    import concourse.bacc as bacc
    import concourse.mybir as mybir
    import concourse.tile as tile
    from contextlib import ExitStack

    f32 = mybir.dt.float32
    f32r = mybir.dt.float32r
    Alu = mybir.AluOpType
    Act = mybir.ActivationFunctionType

    nc = bacc.Bacc("TRN2", debug=False, target_bir_lowering=False, num_devices=B)

    x_d = nc.dram_tensor("x", [L, D], f32r, kind="ExternalInput").ap()
    w1_d = nc.dram_tensor("w1", [KC, P, D], f32r, kind="ExternalInput").ap()
    b1_d = nc.dram_tensor("b1", [P, KC], f32, kind="ExternalInput").ap()
    w2_d = nc.dram_tensor("w2", [KC, P, D], f32r, kind="ExternalInput").ap()
    b2_d = nc.dram_tensor("b2", [P, KC], f32, kind="ExternalInput").ap()
    gw_d = nc.dram_tensor("gw", [GC, P, GC, P], f32r, kind="ExternalInput").ap()
    gb_d = nc.dram_tensor("gb", [P, GC], f32, kind="ExternalInput").ap()
    inv_d = nc.dram_tensor("invsteps", [P, NT], f32, kind="ExternalInput").ap()
    triu_d = nc.dram_tensor("triu", [P, P], f32r, kind="ExternalInput").ap()
    stril_d = nc.dram_tensor("stril", [P, P], f32r, kind="ExternalInput").ap()
    ident_d = nc.dram_tensor("ident", [P, P], f32r, kind="ExternalInput").ap()
    out_d = nc.dram_tensor("out", [L, D], f32, kind="ExternalOutput").ap()
    ffn_d = nc.dram_tensor("ffn", [L, D], f32, kind="ExternalOutput").ap()

    def r(ap):
        return ap.bitcast(f32r)

    def v(ap):
        return ap.bitcast(f32)

    with tile.TileContext(nc) as tc, ExitStack() as ctx:
        consts = ctx.enter_context(tc.tile_pool(name="consts", bufs=1))
        wts = ctx.enter_context(tc.tile_pool(name="wts", bufs=1))
        quart = ctx.enter_context(tc.tile_pool(name="quart", bufs=1))
        xload = ctx.enter_context(tc.tile_pool(name="xload", bufs=2))
        avgp = ctx.enter_context(tc.tile_pool(name="avgp", bufs=2))
        statp = ctx.enter_context(tc.tile_pool(name="statp", bufs=2))
        gwp = ctx.enter_context(tc.tile_pool(name="gwp", bufs=2))
        sigp = ctx.enter_context(tc.tile_pool(name="sigp", bufs=2))
        tmpp = ctx.enter_context(tc.tile_pool(name="tmpp", bufs=2))
        piecep = ctx.enter_context(tc.tile_pool(name="piecep", bufs=4))
        psA_p = ctx.enter_context(tc.tile_pool(name="psA", bufs=1, space="PSUM"))
        trps_p = ctx.enter_context(tc.tile_pool(name="trps", bufs=2, space="PSUM"))
        psB_p = ctx.enter_context(tc.tile_pool(name="psB", bufs=2, space="PSUM"))
        psC_p = ctx.enter_context(tc.tile_pool(name="psC", bufs=1, space="PSUM"))

        triu = consts.tile([P, P], f32r, name="triu_sb")
        nc.sync.dma_start(out=triu, in_=triu_d)
        stril = consts.tile([P, P], f32r, name="stril_sb")
        nc.sync.dma_start(out=stril, in_=stril_d)
        ident = consts.tile([P, P], f32r, name="ident_sb")
        nc.sync.dma_start(out=ident, in_=ident_d)
        inv_sb = consts.tile([P, NT], f32, name="inv_sb")
        nc.sync.dma_start(out=inv_sb, in_=inv_d)
        b1_sb = consts.tile([P, KC], f32, name="b1_sb")
        nc.sync.dma_start(out=b1_sb, in_=b1_d)
        b2_sb = consts.tile([P, KC], f32, name="b2_sb")
        nc.sync.dma_start(out=b2_sb, in_=b2_d)
        gb_sb = consts.tile([P, GC], f32, name="gb_sb")
        nc.sync.dma_start(out=gb_sb, in_=gb_d)
        eps_sb = consts.tile([P, 1], f32, name="eps_sb")
        nc.vector.memset(eps_sb, EPS)

        w1_sb = []
        w2_sb = []
        for k in range(KC):
            t1 = wts.tile([P, D], f32r, name=f"w1sb{k}", tag=f"w1_{k}")
            nc.sync.dma_start(out=t1, in_=w1_d[k])
            w1_sb.append(t1)
            t2 = wts.tile([P, D], f32r, name=f"w2sb{k}", tag=f"w2_{k}")
            nc.sync.dma_start(out=t2, in_=w2_d[k])
            w2_sb.append(t2)

        # persistent PSUM region carrying the running column-sum prefix R
        psA = psA_p.tile([P, D], f32, name="psA_t")

        for q in range(NQ):
            lnT = [quart.tile([P, QW], f32r, name=f"lnT{c}_{q}", tag=f"lnT{c}")
                   for c in range(KC)]
            avgT = [quart.tile([P, QW], f32, name=f"avgT{c}_{q}", tag=f"avgT{c}")
                    for c in range(KC)]
            xT = [quart.tile([P, QW], f32r, name=f"xT{c}_{q}", tag=f"xT{c}")
                  for c in range(KC)]

            # ---- phase A: cumavg + LN + transposes, per 128-token tile ----
            for ti in range(QT):
                i = q * QT + ti
                xi = xload.tile([P, D], f32r, name=f"xi_{i}", tag="xi")
                nc.sync.dma_start(out=xi, in_=x_d[i * P:(i + 1) * P, :])

                # psA += triu-cumsum(x_i)  (now holds R_i + cs_i)
                for s in range(2):
                    nc.tensor.matmul(psA[:, s * 512:(s + 1) * 512], triu,
                                     xi[:, s * 512:(s + 1) * 512],
                                     start=(i == 0), stop=False)
                # avg_i = psA * invsteps_i  (f32r so the transposes can eat it)
                avg_i = avgp.tile([P, D], f32r, name=f"avg_{i}", tag="avg")
                for s in range(2):
                    nc.vector.tensor_scalar_mul(avg_i[:, s * 512:(s + 1) * 512],
                                                psA[:, s * 512:(s + 1) * 512],
                                                inv_sb[:, i:i + 1])
                # psA += strict-lower-tril(x_i)  (now holds R_{i+1})
                for s in range(2):
                    nc.tensor.matmul(psA[:, s * 512:(s + 1) * 512], stril,
                                     xi[:, s * 512:(s + 1) * 512],
                                     start=False, stop=(i == NT - 1))

                # transpose x_i -> xT chunks
                for c in range(KC):
                    pt = trps_p.tile([P, P], f32, name=f"ptx{i}_{c}", tag="tr")
                    nc.tensor.transpose(r(pt), xi[:, c * P:(c + 1) * P], ident)
                    nc.vector.tensor_copy(xT[c][:, ti * P:(ti + 1) * P], pt)

                # LN stats on avg_i
                st6 = statp.tile([P, 12], f32, name=f"st6_{i}", tag="st6")
                nc.vector.bn_stats(st6[:, 0:6], v(avg_i[:, 0:512]))
                nc.vector.bn_stats(st6[:, 6:12], v(avg_i[:, 512:1024]))
                mv = statp.tile([P, 2], f32, name=f"mv_{i}", tag="mv")
                nc.vector.bn_aggr(mv, st6.rearrange("p (g s) -> p g s", g=2))
                std = statp.tile([P, 1], f32, name=f"std_{i}", tag="std")
                nc.scalar.activation(std, mv[:, 1:2], Act.Sqrt, bias=eps_sb)
                rstd = statp.tile([P, 1], f32, name=f"rstd_{i}", tag="rstd")
                nc.vector.reciprocal(rstd, std)

                # transpose avg -> avgT chunks (+ b2 folded in for the residual)
                for c in range(KC):
                    pt = trps_p.tile([P, P], f32, name=f"pta{i}_{c}", tag="tr")
                    nc.tensor.transpose(r(pt), avg_i[:, c * P:(c + 1) * P], ident)
                    nc.scalar.activation(avgT[c][:, ti * P:(ti + 1) * P], pt,
                                         Act.Identity, bias=b2_sb[:, c:c + 1])

                # ln = (avg - mean) * rstd, in place
                nc.vector.tensor_scalar(avg_i, v(avg_i), mv[:, 0:1], rstd,
                                        op0=Alu.subtract, op1=Alu.mult)

                # transpose ln -> lnT chunks
                for c in range(KC):
                    pt = trps_p.tile([P, P], f32, name=f"ptl{i}_{c}", tag="tr")
                    nc.tensor.transpose(r(pt), avg_i[:, c * P:(c + 1) * P], ident)
                    nc.vector.tensor_copy(lnT[c][:, ti * P:(ti + 1) * P], pt)

            # ---- phase B: FFN on this quarter ----
            r1T = [quart.tile([P, QW], f32r, name=f"r1T{c}_{q}", tag=f"r1T{c}")
                   for c in range(KC)]
            for n in range(KC):
                ps = psB_p.tile([P, QW], f32, name=f"ps1_{q}_{n}", tag="psB")
                for k in range(KC):
                    nc.tensor.matmul(ps, w1_sb[k][:, n * P:(n + 1) * P], lnT[k],
                                     start=(k == 0), stop=(k == KC - 1))
                # r1 = max(psum + b1, 0), with f32->f32r rounding on write
                nc.vector.tensor_scalar(r1T[n], ps, b1_sb[:, n:n + 1], 0.0,
                                        op0=Alu.add, op1=Alu.max)

            ffnT = [quart.tile([P, QW], f32r, name=f"ffnT{c}_{q}", tag=f"ffnT{c}")
                    for c in range(KC)]
            for dch in range(KC):
                ps = psB_p.tile([P, QW], f32, name=f"ps2_{q}_{dch}", tag="psB")
                for k in range(KC):
                    nc.tensor.matmul(ps, w2_sb[k][:, dch * P:(dch + 1) * P],
                                     r1T[k], start=(k == 0), stop=(k == KC - 1))
                nc.vector.tensor_tensor(ffnT[dch], ps, avgT[dch], op=Alu.add)
                # ffn output back to natural layout
                for ti in range(QT):
                    pt = trps_p.tile([P, P], f32, name=f"ptf{q}{dch}_{ti}", tag="tr")
                    nc.tensor.transpose(r(pt), ffnT[dch][:, ti * P:(ti + 1) * P],
                                        ident)
                    pf = piecep.tile([P, P], f32, name=f"pf{q}{dch}_{ti}", tag="pf")
                    nc.scalar.copy(pf, pt)
                    t0 = (q * QT + ti) * P
                    nc.sync.dma_start(out=ffn_d[t0:t0 + P, dch * P:(dch + 1) * P],
                                      in_=pf)

            # ---- phase C: gating on this quarter ----
            for j in range(KC):
                gwig = gwp.tile([P, 2 * D], f32r, name=f"gwig_{q}_{j}", tag="gwig")
                nc.sync.dma_start(out=gwig,
                                  in_=gw_d[j].rearrange("p c f -> p (c f)"))
                gwfg = gwp.tile([P, 2 * D], f32r, name=f"gwfg_{q}_{j}", tag="gwfg",
                                bufs=1)
                nc.sync.dma_start(out=gwfg,
                                  in_=gw_d[j + KC].rearrange("p c f -> p (c f)"))

                ps_ig = psC_p.tile([P, QW], f32, name=f"psig_{q}_{j}", tag="ig")
                ps_fg = psC_p.tile([P, QW], f32, name=f"psfg_{q}_{j}", tag="fg")
                for c in range(GC):
                    rhs = xT[c] if c < KC else ffnT[c - KC]
                    nc.tensor.matmul(ps_ig, gwig[:, c * P:(c + 1) * P], rhs,
                                     start=(c == 0), stop=(c == GC - 1))
                for c in range(GC):
                    rhs = xT[c] if c < KC else ffnT[c - KC]
                    nc.tensor.matmul(ps_fg, gwfg[:, c * P:(c + 1) * P], rhs,
                                     start=(c == 0), stop=(c == GC - 1))

                sig_ig = sigp.tile([P, QW], f32, name=f"sigig_{q}_{j}", tag="ig")
                nc.scalar.activation(sig_ig, ps_ig, Act.Sigmoid,
                                     bias=gb_sb[:, j:j + 1])
                sig_fg = sigp.tile([P, QW], f32, name=f"sigfg_{q}_{j}", tag="fg")
                nc.scalar.activation(sig_fg, ps_fg, Act.Sigmoid,
                                     bias=gb_sb[:, j + KC:j + KC + 1])

                a = tmpp.tile([P, QW], f32r, name=f"a_{q}_{j}", tag="a")
                nc.vector.tensor_tensor(a, sig_ig, v(xT[j]), op=Alu.mult)
                nc.vector.tensor_tensor(sig_fg, sig_fg, v(ffnT[j]), op=Alu.mult)
                nc.vector.tensor_tensor(a, v(a), sig_fg, op=Alu.add)

                for ti in range(QT):
                    pt = trps_p.tile([P, P], f32, name=f"pto{q}{j}_{ti}", tag="tr")
                    nc.tensor.transpose(r(pt), a[:, ti * P:(ti + 1) * P], ident)
                    po = piecep.tile([P, P], f32, name=f"po{q}{j}_{ti}", tag="po")
                    nc.vector.tensor_copy(po, pt)
                    t0 = (q * QT + ti) * P
                    nc.sync.dma_start(out=out_d[t0:t0 + P, j * P:(j + 1) * P],
                                      in_=po)

    nc.compile()
    _CACHE["nc"] = nc
    return nc


def _prep_maps(inputs, ln_g, ln_b, w1, b1, w2, b2, gw, gb):
    inputs = np.asarray(inputs, dtype=np.float32)
    ln_g = np.asarray(ln_g, dtype=np.float32)
    ln_b = np.asarray(ln_b, dtype=np.float32)
    w1 = np.asarray(w1, dtype=np.float32)
    b1 = np.asarray(b1, dtype=np.float32)
    w2 = np.asarray(w2, dtype=np.float32)
    b2 = np.asarray(b2, dtype=np.float32)
    gw = np.asarray(gw, dtype=np.float32)
    gb = np.asarray(gb, dtype=np.float32)

    w1f = (ln_g[:, None] * w1).astype(np.float32)
    b1f = (ln_b @ w1 + b1).astype(np.float32)

    base = {
        "w1": np.ascontiguousarray(w1f.reshape(KC, P, D)),
        "b1": np.ascontiguousarray(b1f.reshape(KC, P).T),
        "w2": np.ascontiguousarray(w2.reshape(KC, P, D)),
        "b2": np.ascontiguousarray(b2.reshape(KC, P).T),
        "gw": np.ascontiguousarray(
            gw.reshape(GC, P, GC, P).transpose(2, 1, 0, 3)),
        "gb": np.ascontiguousarray(gb.reshape(GC, P).T),
        "invsteps": np.ascontiguousarray(
            (1.0 / np.arange(1, L + 1, dtype=np.float32)).reshape(NT, P).T),
        "triu": np.triu(np.ones((P, P), np.float32)),
        "stril": np.tril(np.ones((P, P), np.float32), -1),
        "ident": np.eye(P, dtype=np.float32),
    }
    return [dict(base, x=np.ascontiguousarray(inputs[b])) for b in range(B)]


def _run(in_maps, trace=False):
    from concourse.bass_utils import run_bass_kernel_spmd
    nc = _build()
    return run_bass_kernel_spmd(nc, in_maps, list(range(B)), trace=trace)


def kernel(inputs, ln_g, ln_b, w1, b1, w2, b2, gw, gb):
    in_maps = _prep_maps(inputs, ln_g, ln_b, w1, b1, w2, b2, gw, gb)
    res = _run(in_maps).results
    out = np.stack([res[b]["out"] for b in range(B)])
    ffn = np.stack([res[b]["ffn"] for b in range(B)])
    return out, ffn


def kernel_traced(inputs, ln_g, ln_b, w1, b1, w2, b2, gw, gb):
    """Like kernel(), but also returns the BassKernelResults (with exec_time_ns)."""
    in_maps = _prep_maps(inputs, ln_g, ln_b, w1, b1, w2, b2, gw, gb)
    bkr = _run(in_maps, trace=True)
    res = bkr.results
    out = np.stack([res[b]["out"] for b in range(B)])
    ffn = np.stack([res[b]["ffn"] for b in range(B)])
    return (out, ffn), bkr


# revision 11
# speedup vs baseline: 1.3801x; 1.3801x over previous
"""Bass/Tile TRN2 kernel for nn_AverageAttention (cumavg -> LN -> FFN -> sigmoid gating).

Sharding: data-parallel over batch, one batch element per NeuronCore (B=8, 8 cores).

Per-core pipeline (L=2048 tokens processed in 4 quarters of 512 tokens = 4 tiles
of 128):
  phase A (per 128-token tile, natural [t, d] layout):
     cumavg via triu-matmul + running-prefix (strict-lower-tril matmul) in a
     persistent PSUM region; LayerNorm stats via bn_stats/bn_aggr; PE-transposes
     (batched 4-per-PSUM-bank, single strided evac) produce xT / avgT / lnT
     chunks in [d, t] layout.
  phase B (per quarter): y1T = w1'@lnT (relu, +b1'), y2T = w2@r1T,
     ffnT = y2T + b2 + avgT (one scalar_tensor_tensor); ffn transposed back to
     natural and written out.
  phase C (per quarter): gating gT = gw@[xT; ffnT] in BF16 (gw streamed per
     128-col slice), sigmoid (+gb), outT = sig_ig*xT + sig_fg*ffnT (fp32),
     transposed back and written out.

ln_g/ln_b are folded into w1/b1 on the host.  FFN/cumsum matmuls run as
float32r (2-pass fp32); the gating matmul runs in bf16 (its error is damped by
the sigmoid), reading bf16 shadow copies xTb/ffnTb while the fp32 xT/ffnT feed
the exact final elementwise.
"""

import numpy as np

B, L, D = 8, 2048, 1024
P = 128
NT = L // P          # 16 token tiles
KC = D // P          # 8 d-chunks
GC = 2 * D // P      # 16 gating chunks
QT = 4               # tiles per quarter
NQ = NT // QT        # 4 quarters
QW = QT * P          # 512 tokens per quarter
EPS = 1e-6

_CACHE = {}


def _build():
    if "nc" in _CACHE:
        return _CACHE["nc"]

    import concourse.bacc as bacc
    import concourse.mybir as mybir
    import concourse.tile as tile
    from contextlib import ExitStack

    f32 = mybir.dt.float32
    f32r = mybir.dt.float32r
    bf16 = mybir.dt.bfloat16
    Alu = mybir.AluOpType
    Act = mybir.ActivationFunctionType

    nc = bacc.Bacc("TRN2", debug=False, target_bir_lowering=False, num_devices=B)

    x_d = nc.dram_tensor("x", [L, D], f32r, kind="ExternalInput").ap()
    w1_d = nc.dram_tensor("w1", [KC, P, D], f32r, kind="ExternalInput").ap()
    b1_d = nc.dram_tensor("b1", [P, KC], f32, kind="ExternalInput").ap()
    w2_d = nc.dram_tensor("w2", [KC, P, D], f32r, kind="ExternalInput").ap()
    b2_d = nc.dram_tensor("b2", [P, KC], f32, kind="ExternalInput").ap()
    gw_d = nc.dram_tensor("gw", [GC, P, GC, P], bf16, kind="ExternalInput").ap()
    gb_d = nc.dram_tensor("gb", [P, GC], f32, kind="ExternalInput").ap()
    inv_d = nc.dram_tensor("invsteps", [P, NT], f32, kind="ExternalInput").ap()
    triu_d = nc.dram_tensor("triu", [P, P], f32r, kind="ExternalInput").ap()
    stril_d = nc.dram_tensor("stril", [P, P], f32r, kind="ExternalInput").ap()
    ident_d = nc.dram_tensor("ident", [P, P], f32r, kind="ExternalInput").ap()
    out_d = nc.dram_tensor("out", [L, D], f32, kind="ExternalOutput").ap()
    ffn_d = nc.dram_tensor("ffn", [L, D], f32, kind="ExternalOutput").ap()

    def r(ap):
        return ap.bitcast(f32r)

    def v(ap):
        return ap.bitcast(f32)

    def wide3(ap, inner=P):
        # [P, KC*QW] wide tile viewed as [P, nblk, inner]
        return ap.rearrange("p (b t) -> p b t", t=inner)

    with tile.TileContext(nc) as tc, ExitStack() as ctx:
        consts = ctx.enter_context(tc.tile_pool(name="consts", bufs=1))
        wts = ctx.enter_context(tc.tile_pool(name="wts", bufs=1))
        quart = ctx.enter_context(tc.tile_pool(name="quart", bufs=1))
        xload = ctx.enter_context(tc.tile_pool(name="xload", bufs=2))
        avgp = ctx.enter_context(tc.tile_pool(name="avgp", bufs=2))
        statp = ctx.enter_context(tc.tile_pool(name="statp", bufs=2))
        gwp = ctx.enter_context(tc.tile_pool(name="gwp", bufs=2))
        sigp = ctx.enter_context(tc.tile_pool(name="sigp", bufs=1))
        tmpp = ctx.enter_context(tc.tile_pool(name="tmpp", bufs=2))
        piecep = ctx.enter_context(tc.tile_pool(name="piecep", bufs=1))
        psA_p = ctx.enter_context(tc.tile_pool(name="psA", bufs=1, space="PSUM"))
        trps_p = ctx.enter_context(tc.tile_pool(name="trps", bufs=2, space="PSUM"))
        psB_p = ctx.enter_context(tc.tile_pool(name="psB", bufs=2, space="PSUM"))
        psC_p = ctx.enter_context(tc.tile_pool(name="psC", bufs=1, space="PSUM"))

        triu = consts.tile([P, P], f32r, name="triu_sb")
        nc.sync.dma_start(out=triu, in_=triu_d)
        stril = consts.tile([P, P], f32r, name="stril_sb")
        nc.sync.dma_start(out=stril, in_=stril_d)
        ident = consts.tile([P, P], f32r, name="ident_sb")
        nc.sync.dma_start(out=ident, in_=ident_d)
        inv_sb = consts.tile([P, NT], f32, name="inv_sb")
        nc.sync.dma_start(out=inv_sb, in_=inv_d)
        b1_sb = consts.tile([P, KC], f32, name="b1_sb")
        nc.sync.dma_start(out=b1_sb, in_=b1_d)
        b2_sb = consts.tile([P, KC], f32, name="b2_sb")
        nc.sync.dma_start(out=b2_sb, in_=b2_d)
        gb_sb = consts.tile([P, GC], f32, name="gb_sb")
        nc.sync.dma_start(out=gb_sb, in_=gb_d)
        eps_sb = consts.tile([P, 1], f32, name="eps_sb")
        nc.vector.memset(eps_sb, EPS)

        # weights on the ACT HWDGE queue so phase A's x loads go first on sync
        w1_sb = []
        w2_sb = []
        for k in range(KC):
            t1 = wts.tile([P, D], f32r, name=f"w1sb{k}", tag=f"w1_{k}")
            nc.scalar.dma_start(out=t1, in_=w1_d[k])
            w1_sb.append(t1)
            t2 = wts.tile([P, D], f32r, name=f"w2sb{k}", tag=f"w2_{k}")
            nc.scalar.dma_start(out=t2, in_=w2_d[k])
            w2_sb.append(t2)

        # persistent PSUM region carrying the running column-sum prefix R
        psA = psA_p.tile([P, D], f32, name="psA_t")

        for q in range(NQ):
            lnT = quart.tile([P, KC * QW], f32r, name=f"lnT_{q}", tag="lnT")
            avgT = quart.tile([P, KC * QW], f32, name=f"avgT_{q}", tag="avgT")
            xT = quart.tile([P, KC * QW], f32r, name=f"xT_{q}", tag="xT")
            xTb = quart.tile([P, KC * QW], bf16, name=f"xTb_{q}", tag="xTb")

            # ---- phase A: cumavg + LN + transposes, per 128-token tile ----
            for ti in range(QT):
                i = q * QT + ti
                xi = xload.tile([P, D], f32r, name=f"xi_{i}", tag="xi")
                nc.sync.dma_start(out=xi, in_=x_d[i * P:(i + 1) * P, :])

                # psA += triu-cumsum(x_i)  (now holds R_i + cs_i)
                for s in range(2):
                    nc.tensor.matmul(psA[:, s * 512:(s + 1) * 512], triu,
                                     xi[:, s * 512:(s + 1) * 512],
                                     start=(i == 0), stop=False)
                # avg_i = psA * invsteps_i  (f32r so the transposes can eat it)
                avg_i = avgp.tile([P, D], f32r, name=f"avg_{i}", tag="avg")
                for s in range(2):
                    nc.vector.tensor_scalar_mul(avg_i[:, s * 512:(s + 1) * 512],
                                                psA[:, s * 512:(s + 1) * 512],
                                                inv_sb[:, i:i + 1])
                # psA += strict-lower-tril(x_i)  (now holds R_{i+1})
                for s in range(2):
                    nc.tensor.matmul(psA[:, s * 512:(s + 1) * 512], stril,
                                     xi[:, s * 512:(s + 1) * 512],
                                     start=False, stop=(i == NT - 1))

                # transpose x_i -> xT chunks (batched 4 per PSUM bank), plus a
                # bf16 shadow copy for the gating matmul
                for g in range(2):
                    pt = trps_p.tile([P, 512], f32, name=f"ptx{i}_{g}", tag="tr")
                    for cc in range(4):
                        c = g * 4 + cc
                        nc.tensor.transpose(r(pt[:, cc * P:(cc + 1) * P]),
                                            xi[:, c * P:(c + 1) * P], ident)
                    dst = wide3(xT, QW)[:, g * 4:(g + 1) * 4, ti * P:(ti + 1) * P]
                    nc.vector.tensor_copy(dst, wide3(pt))
                    dstb = wide3(xTb, QW)[:, g * 4:(g + 1) * 4, ti * P:(ti + 1) * P]
                    nc.vector.tensor_copy(dstb, wide3(pt))

                # LN stats on avg_i
                st6 = statp.tile([P, 12], f32, name=f"st6_{i}", tag="st6")
                nc.vector.bn_stats(st6[:, 0:6], v(avg_i[:, 0:512]))
                nc.vector.bn_stats(st6[:, 6:12], v(avg_i[:, 512:1024]))
                mv = statp.tile([P, 2], f32, name=f"mv_{i}", tag="mv")
                nc.vector.bn_aggr(mv, st6.rearrange("p (g s) -> p g s", g=2))
                std = statp.tile([P, 1], f32, name=f"std_{i}", tag="std")
                nc.scalar.activation(std, mv[:, 1:2], Act.Sqrt, bias=eps_sb)
                rstd = statp.tile([P, 1], f32, name=f"rstd_{i}", tag="rstd")
                nc.vector.reciprocal(rstd, std)

                # transpose avg -> avgT chunks (batched)
                for g in range(2):
                    pt = trps_p.tile([P, 512], f32, name=f"pta{i}_{g}", tag="tr")
                    for cc in range(4):
                        c = g * 4 + cc
                        nc.tensor.transpose(r(pt[:, cc * P:(cc + 1) * P]),
                                            avg_i[:, c * P:(c + 1) * P], ident)
                    dst = wide3(avgT, QW)[:, g * 4:(g + 1) * 4, ti * P:(ti + 1) * P]
                    nc.scalar.copy(dst, wide3(pt))

                # ln = (avg - mean) * rstd, in place
                nc.vector.tensor_scalar(avg_i, v(avg_i), mv[:, 0:1], rstd,
                                        op0=Alu.subtract, op1=Alu.mult)

                # transpose ln -> lnT chunks (batched)
                for g in range(2):
                    pt = trps_p.tile([P, 512], f32, name=f"ptl{i}_{g}", tag="tr")
                    for cc in range(4):
                        c = g * 4 + cc
                        nc.tensor.transpose(r(pt[:, cc * P:(cc + 1) * P]),
                                            avg_i[:, c * P:(c + 1) * P], ident)
                    dst = wide3(lnT, QW)[:, g * 4:(g + 1) * 4, ti * P:(ti + 1) * P]
                    nc.vector.tensor_copy(dst, wide3(pt))

            # ---- phase B: FFN on this quarter ----
            r1T = quart.tile([P, KC * QW], f32r, name=f"r1T_{q}", tag="r1T")
            for n in range(KC):
                ps = psB_p.tile([P, QW], f32, name=f"ps1_{q}_{n}", tag="psB")
                for k in range(KC):
                    nc.tensor.matmul(ps, w1_sb[k][:, n * P:(n + 1) * P],
                                     lnT[:, k * QW:(k + 1) * QW],
                                     start=(k == 0), stop=(k == KC - 1))
                # r1 = max(psum + b1, 0), f32->f32r rounding on write
                nc.vector.tensor_scalar(r1T[:, n * QW:(n + 1) * QW], ps,
                                        b1_sb[:, n:n + 1], 0.0,
                                        op0=Alu.add, op1=Alu.max)

            ffnT = quart.tile([P, KC * QW], f32r, name=f"ffnT_{q}", tag="ffnT")
            ffnTb = quart.tile([P, KC * QW], bf16, name=f"ffnTb_{q}", tag="ffnTb")
            for dch in range(KC):
                ps = psB_p.tile([P, QW], f32, name=f"ps2_{q}_{dch}", tag="psB")
                for k in range(KC):
                    nc.tensor.matmul(ps, w2_sb[k][:, dch * P:(dch + 1) * P],
                                     r1T[:, k * QW:(k + 1) * QW],
                                     start=(k == 0), stop=(k == KC - 1))
                # ffnT = (y2T + b2) + avgT
                nc.vector.scalar_tensor_tensor(
                    ffnT[:, dch * QW:(dch + 1) * QW], ps, b2_sb[:, dch:dch + 1],
                    avgT[:, dch * QW:(dch + 1) * QW], op0=Alu.add, op1=Alu.add)
                nc.vector.tensor_copy(ffnTb[:, dch * QW:(dch + 1) * QW],
                                      v(ffnT[:, dch * QW:(dch + 1) * QW]))
                # ffn output back to natural layout (batched transposes + 1 DMA)
                pt = trps_p.tile([P, 512], f32, name=f"ptf{q}_{dch}", tag="tr")
                for ti in range(QT):
                    nc.tensor.transpose(
                        r(pt[:, ti * P:(ti + 1) * P]),
                        ffnT[:, dch * QW + ti * P:dch * QW + (ti + 1) * P],
                        ident)
                pf = piecep.tile([P, 512], f32, name=f"pf{q}_{dch}", tag="pf")
                nc.scalar.copy(pf, pt)
                t0 = q * QW
                dram = ffn_d[t0:t0 + QW, dch * P:(dch + 1) * P]
                nc.sync.dma_start(out=dram.rearrange("(b p) f -> p b f", p=P),
                                  in_=wide3(pf))

            # ---- phase C: gating on this quarter (bf16 matmuls) ----
            for j in range(KC):
                gwig = gwp.tile([P, 2 * D], bf16, name=f"gwig_{q}_{j}", tag="gwig")
                nc.sync.dma_start(out=gwig,
                                  in_=gw_d[j].rearrange("p c f -> p (c f)"))
                gwfg = gwp.tile([P, 2 * D], bf16, name=f"gwfg_{q}_{j}", tag="gwfg")
                nc.sync.dma_start(out=gwfg,
                                  in_=gw_d[j + KC].rearrange("p c f -> p (c f)"))

                ps_ig = psC_p.tile([P, QW], f32, name=f"psig_{q}_{j}", tag="ig")
                ps_fg = psC_p.tile([P, QW], f32, name=f"psfg_{q}_{j}", tag="fg")
                for c in range(GC):
                    rhs = (xTb[:, c * QW:(c + 1) * QW] if c < KC else
                           ffnTb[:, (c - KC) * QW:(c - KC + 1) * QW])
                    nc.tensor.matmul(ps_ig, gwig[:, c * P:(c + 1) * P], rhs,
                                     start=(c == 0), stop=(c == GC - 1))
                for c in range(GC):
                    rhs = (xTb[:, c * QW:(c + 1) * QW] if c < KC else
                           ffnTb[:, (c - KC) * QW:(c - KC + 1) * QW])
                    nc.tensor.matmul(ps_fg, gwfg[:, c * P:(c + 1) * P], rhs,
                                     start=(c == 0), stop=(c == GC - 1))

                sig_ig = sigp.tile([P, QW], f32, name=f"sigig_{q}_{j}", tag="ig")
                nc.scalar.activation(sig_ig, ps_ig, Act.Sigmoid,
                                     bias=gb_sb[:, j:j + 1])
                sig_fg = sigp.tile([P, QW], f32, name=f"sigfg_{q}_{j}", tag="fg")
                nc.scalar.activation(sig_fg, ps_fg, Act.Sigmoid,
                                     bias=gb_sb[:, j + KC:j + KC + 1])

                a = tmpp.tile([P, QW], f32r, name=f"a_{q}_{j}", tag="a")
                nc.vector.tensor_tensor(a, sig_ig, v(xT[:, j * QW:(j + 1) * QW]),
                                        op=Alu.mult)
                nc.vector.tensor_tensor(sig_fg, sig_fg,
                                        v(ffnT[:, j * QW:(j + 1) * QW]), op=Alu.mult)
                nc.vector.tensor_tensor(a, v(a), sig_fg, op=Alu.add)

                pt = trps_p.tile([P, 512], f32, name=f"pto{q}_{j}", tag="tr")
                for ti in range(QT):
                    nc.tensor.transpose(r(pt[:, ti * P:(ti + 1) * P]),
                                        a[:, ti * P:(ti + 1) * P], ident)
                po = piecep.tile([P, 512], f32, name=f"po{q}_{j}", tag="po")
                nc.vector.tensor_copy(po, pt)
                t0 = q * QW
                dram = out_d[t0:t0 + QW, j * P:(j + 1) * P]
                nc.sync.dma_start(out=dram.rearrange("(b p) f -> p b f", p=P),
                                  in_=wide3(po))

    nc.compile()
    _CACHE["nc"] = nc
    return nc


def _prep_maps(inputs, ln_g, ln_b, w1, b1, w2, b2, gw, gb):
    import ml_dtypes

    inputs = np.asarray(inputs, dtype=np.float32)
    ln_g = np.asarray(ln_g, dtype=np.float32)
    ln_b = np.asarray(ln_b, dtype=np.float32)
    w1 = np.asarray(w1, dtype=np.float32)
    b1 = np.asarray(b1, dtype=np.float32)
    w2 = np.asarray(w2, dtype=np.float32)
    b2 = np.asarray(b2, dtype=np.float32)
    gw = np.asarray(gw, dtype=np.float32)
    gb = np.asarray(gb, dtype=np.float32)

    w1f = (ln_g[:, None] * w1).astype(np.float32)
    b1f = (ln_b @ w1 + b1).astype(np.float32)

    base = {
        "w1": np.ascontiguousarray(w1f.reshape(KC, P, D)),
        "b1": np.ascontiguousarray(b1f.reshape(KC, P).T),
        "w2": np.ascontiguousarray(w2.reshape(KC, P, D)),
        "b2": np.ascontiguousarray(b2.reshape(KC, P).T),
        "gw": np.ascontiguousarray(
            gw.reshape(GC, P, GC, P).transpose(2, 1, 0, 3)).astype(
                ml_dtypes.bfloat16),
        "gb": np.ascontiguousarray(gb.reshape(GC, P).T),
        "invsteps": np.ascontiguousarray(
            (1.0 / np.arange(1, L + 1, dtype=np.float32)).reshape(NT, P).T),
        "triu": np.triu(np.ones((P, P), np.float32)),
        "stril": np.tril(np.ones((P, P), np.float32), -1),
        "ident": np.eye(P, dtype=np.float32),
    }
    return [dict(base, x=np.ascontiguousarray(inputs[b])) for b in range(B)]


def _run(in_maps, trace=False):
    from concourse.bass_utils import run_bass_kernel_spmd
    nc = _build()
    return run_bass_kernel_spmd(nc, in_maps, list(range(B)), trace=trace)


def kernel(inputs, ln_g, ln_b, w1, b1, w2, b2, gw, gb):
    in_maps = _prep_maps(inputs, ln_g, ln_b, w1, b1, w2, b2, gw, gb)
    res = _run(in_maps).results
    out = np.stack([res[b]["out"] for b in range(B)])
    ffn = np.stack([res[b]["ffn"] for b in range(B)])
    return out, ffn


def kernel_traced(inputs, ln_g, ln_b, w1, b1, w2, b2, gw, gb):
    """Like kernel(), but also returns the BassKernelResults (with exec_time_ns)."""
    in_maps = _prep_maps(inputs, ln_g, ln_b, w1, b1, w2, b2, gw, gb)
    bkr = _run(in_maps, trace=True)
    res = bkr.results
    out = np.stack([res[b]["out"] for b in range(B)])
    ffn = np.stack([res[b]["ffn"] for b in range(B)])
    return (out, ffn), bkr


# revision 12
# speedup vs baseline: 1.4104x; 1.0219x over previous
"""Bass/Tile TRN2 kernel for nn_AverageAttention (cumavg -> LN -> FFN -> sigmoid gating).

Sharding: data-parallel over batch, one batch element per NeuronCore (B=8, 8 cores).

Per-core pipeline (L=2048 tokens processed in 4 quarters of 512 tokens = 4 tiles
of 128):
  phase A (per 128-token tile, natural [t, d] layout):
     cumavg via triu-matmul + running-prefix (strict-lower-tril matmul) in a
     persistent PSUM region; LayerNorm stats via bn_stats/bn_aggr; PE-transposes
     (batched 4-per-PSUM-bank, single strided evac) produce xT / avgT / lnT
     chunks in [d, t] layout.
  phase B (per quarter): y1T = w1'@lnT (relu, +b1'), y2T = w2@r1T,
     ffnT = y2T + b2 + avgT (one scalar_tensor_tensor); ffn transposed back to
     natural and written out.
  phase C (per quarter): gating gT = gw@[xT; ffnT] in BF16 (gw streamed per
     128-col slice), sigmoid (+gb), outT = sig_ig*xT + sig_fg*ffnT (fp32),
     transposed back and written out.

ln_g/ln_b are folded into w1/b1 on the host.  FFN/cumsum matmuls run as
float32r (2-pass fp32); the gating matmul runs in bf16 (its error is damped by
the sigmoid), reading bf16 shadow copies xTb/ffnTb while the fp32 xT/ffnT feed
the exact final elementwise.
"""

import numpy as np

B, L, D = 8, 2048, 1024
P = 128
NT = L // P          # 16 token tiles
KC = D // P          # 8 d-chunks
GC = 2 * D // P      # 16 gating chunks
QT = 4               # tiles per quarter
NQ = NT // QT        # 4 quarters
QW = QT * P          # 512 tokens per quarter
EPS = 1e-6

_CACHE = {}


def _build():
    if "nc" in _CACHE:
        return _CACHE["nc"]

    import concourse.bacc as bacc
    import concourse.mybir as mybir
    import concourse.tile as tile
    from contextlib import ExitStack

    f32 = mybir.dt.float32
    f32r = mybir.dt.float32r
    bf16 = mybir.dt.bfloat16
    f16 = mybir.dt.float16
    Alu = mybir.AluOpType
    Act = mybir.ActivationFunctionType

    nc = bacc.Bacc("TRN2", debug=False, target_bir_lowering=False, num_devices=B)

    x_d = nc.dram_tensor("x", [L, D], f32r, kind="ExternalInput").ap()
    w1_d = nc.dram_tensor("w1", [KC, P, D], f16, kind="ExternalInput").ap()
    b1_d = nc.dram_tensor("b1", [P, KC], f32, kind="ExternalInput").ap()
    w2_d = nc.dram_tensor("w2", [KC, P, D], f16, kind="ExternalInput").ap()
    b2_d = nc.dram_tensor("b2", [P, KC], f32, kind="ExternalInput").ap()
    gw_d = nc.dram_tensor("gw", [GC, P, GC, P], f16, kind="ExternalInput").ap()
    gb_d = nc.dram_tensor("gb", [P, GC], f32, kind="ExternalInput").ap()
    inv_d = nc.dram_tensor("invsteps", [P, NT], f32, kind="ExternalInput").ap()
    triu_d = nc.dram_tensor("triu", [P, P], f32r, kind="ExternalInput").ap()
    stril_d = nc.dram_tensor("stril", [P, P], f32r, kind="ExternalInput").ap()
    ident_d = nc.dram_tensor("ident", [P, P], f32r, kind="ExternalInput").ap()
    out_d = nc.dram_tensor("out", [L, D], f32, kind="ExternalOutput").ap()
    ffn_d = nc.dram_tensor("ffn", [L, D], f32, kind="ExternalOutput").ap()

    def r(ap):
        return ap.bitcast(f32r)

    def v(ap):
        return ap.bitcast(f32)

    def wide3(ap, inner=P):
        # [P, KC*QW] wide tile viewed as [P, nblk, inner]
        return ap.rearrange("p (b t) -> p b t", t=inner)

    with tile.TileContext(nc) as tc, ExitStack() as ctx:
        consts = ctx.enter_context(tc.tile_pool(name="consts", bufs=1))
        wts = ctx.enter_context(tc.tile_pool(name="wts", bufs=1))
        quart = ctx.enter_context(tc.tile_pool(name="quart", bufs=1))
        xload = ctx.enter_context(tc.tile_pool(name="xload", bufs=2))
        avgp = ctx.enter_context(tc.tile_pool(name="avgp", bufs=2))
        statp = ctx.enter_context(tc.tile_pool(name="statp", bufs=2))
        gwp = ctx.enter_context(tc.tile_pool(name="gwp", bufs=2))
        sigp = ctx.enter_context(tc.tile_pool(name="sigp", bufs=1))
        tmpp = ctx.enter_context(tc.tile_pool(name="tmpp", bufs=2))
        piecep = ctx.enter_context(tc.tile_pool(name="piecep", bufs=1))
        psA_p = ctx.enter_context(tc.tile_pool(name="psA", bufs=1, space="PSUM"))
        trps_p = ctx.enter_context(tc.tile_pool(name="trps", bufs=2, space="PSUM"))
        psB_p = ctx.enter_context(tc.tile_pool(name="psB", bufs=2, space="PSUM"))
        psC_p = ctx.enter_context(tc.tile_pool(name="psC", bufs=1, space="PSUM"))

        triu = consts.tile([P, P], f32r, name="triu_sb")
        nc.sync.dma_start(out=triu, in_=triu_d)
        stril = consts.tile([P, P], f32r, name="stril_sb")
        nc.sync.dma_start(out=stril, in_=stril_d)
        ident = consts.tile([P, P], f32r, name="ident_sb")
        nc.sync.dma_start(out=ident, in_=ident_d)
        inv_sb = consts.tile([P, NT], f32, name="inv_sb")
        nc.sync.dma_start(out=inv_sb, in_=inv_d)
        b1_sb = consts.tile([P, KC], f32, name="b1_sb")
        nc.sync.dma_start(out=b1_sb, in_=b1_d)
        b2_sb = consts.tile([P, KC], f32, name="b2_sb")
        nc.sync.dma_start(out=b2_sb, in_=b2_d)
        gb_sb = consts.tile([P, GC], f32, name="gb_sb")
        nc.sync.dma_start(out=gb_sb, in_=gb_d)
        eps_sb = consts.tile([P, 1], f32, name="eps_sb")
        nc.vector.memset(eps_sb, EPS)

        # weights on the ACT HWDGE queue so phase A's x loads go first on sync
        w1_sb = []
        w2_sb = []
        for k in range(KC):
            t1 = wts.tile([P, D], f16, name=f"w1sb{k}", tag=f"w1_{k}")
            nc.scalar.dma_start(out=t1, in_=w1_d[k])
            w1_sb.append(t1)
            t2 = wts.tile([P, D], f16, name=f"w2sb{k}", tag=f"w2_{k}")
            nc.scalar.dma_start(out=t2, in_=w2_d[k])
            w2_sb.append(t2)

        # persistent PSUM region carrying the running column-sum prefix R
        psA = psA_p.tile([P, D], f32, name="psA_t")

        for q in range(NQ):
            lnT = quart.tile([P, KC * QW], f16, name=f"lnT_{q}", tag="lnT")
            avgT = quart.tile([P, KC * QW], f32, name=f"avgT_{q}", tag="avgT")
            xT = quart.tile([P, KC * QW], f32r, name=f"xT_{q}", tag="xT")
            xTb = quart.tile([P, KC * QW], f16, name=f"xTb_{q}", tag="xTb")

            # ---- phase A: cumavg + LN + transposes, per 128-token tile ----
            for ti in range(QT):
                i = q * QT + ti
                xi = xload.tile([P, D], f32r, name=f"xi_{i}", tag="xi")
                nc.sync.dma_start(out=xi, in_=x_d[i * P:(i + 1) * P, :])

                # psA += triu-cumsum(x_i)  (now holds R_i + cs_i)
                for s in range(2):
                    nc.tensor.matmul(psA[:, s * 512:(s + 1) * 512], triu,
                                     xi[:, s * 512:(s + 1) * 512],
                                     start=(i == 0), stop=False)
                # avg_i = psA * invsteps_i  (f32r so the transposes can eat it)
                avg_i = avgp.tile([P, D], f32r, name=f"avg_{i}", tag="avg")
                for s in range(2):
                    nc.vector.tensor_scalar_mul(avg_i[:, s * 512:(s + 1) * 512],
                                                psA[:, s * 512:(s + 1) * 512],
                                                inv_sb[:, i:i + 1])
                # psA += strict-lower-tril(x_i)  (now holds R_{i+1})
                for s in range(2):
                    nc.tensor.matmul(psA[:, s * 512:(s + 1) * 512], stril,
                                     xi[:, s * 512:(s + 1) * 512],
                                     start=False, stop=(i == NT - 1))

                # transpose x_i -> xT chunks (batched 4 per PSUM bank), plus a
                # bf16 shadow copy for the gating matmul
                for g in range(2):
                    pt = trps_p.tile([P, 512], f32, name=f"ptx{i}_{g}", tag="tr")
                    for cc in range(4):
                        c = g * 4 + cc
                        nc.tensor.transpose(r(pt[:, cc * P:(cc + 1) * P]),
                                            xi[:, c * P:(c + 1) * P], ident)
                    dst = wide3(xT, QW)[:, g * 4:(g + 1) * 4, ti * P:(ti + 1) * P]
                    nc.vector.tensor_copy(dst, wide3(pt))
                    dstb = wide3(xTb, QW)[:, g * 4:(g + 1) * 4, ti * P:(ti + 1) * P]
                    nc.vector.tensor_copy(dstb, wide3(pt))

                # LN stats on avg_i
                st6 = statp.tile([P, 12], f32, name=f"st6_{i}", tag="st6")
                nc.vector.bn_stats(st6[:, 0:6], v(avg_i[:, 0:512]))
                nc.vector.bn_stats(st6[:, 6:12], v(avg_i[:, 512:1024]))
                mv = statp.tile([P, 2], f32, name=f"mv_{i}", tag="mv")
                nc.vector.bn_aggr(mv, st6.rearrange("p (g s) -> p g s", g=2))
                std = statp.tile([P, 1], f32, name=f"std_{i}", tag="std")
                nc.scalar.activation(std, mv[:, 1:2], Act.Sqrt, bias=eps_sb)
                rstd = statp.tile([P, 1], f32, name=f"rstd_{i}", tag="rstd")
                nc.vector.reciprocal(rstd, std)

                # transpose avg -> avgT chunks (batched)
                for g in range(2):
                    pt = trps_p.tile([P, 512], f32, name=f"pta{i}_{g}", tag="tr")
                    for cc in range(4):
                        c = g * 4 + cc
                        nc.tensor.transpose(r(pt[:, cc * P:(cc + 1) * P]),
                                            avg_i[:, c * P:(c + 1) * P], ident)
                    dst = wide3(avgT, QW)[:, g * 4:(g + 1) * 4, ti * P:(ti + 1) * P]
                    nc.scalar.copy(dst, wide3(pt))

                # ln = (avg - mean) * rstd, in place
                nc.vector.tensor_scalar(avg_i, v(avg_i), mv[:, 0:1], rstd,
                                        op0=Alu.subtract, op1=Alu.mult)

                # transpose ln -> lnT chunks (batched)
                for g in range(2):
                    pt = trps_p.tile([P, 512], f32, name=f"ptl{i}_{g}", tag="tr")
                    for cc in range(4):
                        c = g * 4 + cc
                        nc.tensor.transpose(r(pt[:, cc * P:(cc + 1) * P]),
                                            avg_i[:, c * P:(c + 1) * P], ident)
                    dst = wide3(lnT, QW)[:, g * 4:(g + 1) * 4, ti * P:(ti + 1) * P]
                    nc.vector.tensor_copy(dst, wide3(pt))

            # ---- phase B: FFN on this quarter ----
            r1T = quart.tile([P, KC * QW], f16, name=f"r1T_{q}", tag="r1T")
            for n in range(KC):
                ps = psB_p.tile([P, QW], f32, name=f"ps1_{q}_{n}", tag="psB")
                for k in range(KC):
                    nc.tensor.matmul(ps, w1_sb[k][:, n * P:(n + 1) * P],
                                     lnT[:, k * QW:(k + 1) * QW],
                                     start=(k == 0), stop=(k == KC - 1))
                # r1 = max(psum + b1, 0), f32->f32r rounding on write
                nc.vector.tensor_scalar(r1T[:, n * QW:(n + 1) * QW], ps,
                                        b1_sb[:, n:n + 1], 0.0,
                                        op0=Alu.add, op1=Alu.max)

            ffnT = quart.tile([P, KC * QW], f32r, name=f"ffnT_{q}", tag="ffnT")
            ffnTb = quart.tile([P, KC * QW], f16, name=f"ffnTb_{q}", tag="ffnTb")
            for dch in range(KC):
                ps = psB_p.tile([P, QW], f32, name=f"ps2_{q}_{dch}", tag="psB")
                for k in range(KC):
                    nc.tensor.matmul(ps, w2_sb[k][:, dch * P:(dch + 1) * P],
                                     r1T[:, k * QW:(k + 1) * QW],
                                     start=(k == 0), stop=(k == KC - 1))
                # ffnT = (y2T + b2) + avgT
                nc.vector.scalar_tensor_tensor(
                    ffnT[:, dch * QW:(dch + 1) * QW], ps, b2_sb[:, dch:dch + 1],
                    avgT[:, dch * QW:(dch + 1) * QW], op0=Alu.add, op1=Alu.add)
                nc.vector.tensor_copy(ffnTb[:, dch * QW:(dch + 1) * QW],
                                      v(ffnT[:, dch * QW:(dch + 1) * QW]))
                # ffn output back to natural layout (batched transposes + 1 DMA)
                pt = trps_p.tile([P, 512], f32, name=f"ptf{q}_{dch}", tag="tr")
                for ti in range(QT):
                    nc.tensor.transpose(
                        r(pt[:, ti * P:(ti + 1) * P]),
                        ffnT[:, dch * QW + ti * P:dch * QW + (ti + 1) * P],
                        ident)
                pf = piecep.tile([P, 512], f32, name=f"pf{q}_{dch}", tag="pf")
                nc.scalar.copy(pf, pt)
                t0 = q * QW
                dram = ffn_d[t0:t0 + QW, dch * P:(dch + 1) * P]
                nc.sync.dma_start(out=dram.rearrange("(b p) f -> p b f", p=P),
                                  in_=wide3(pf))

            # ---- phase C: gating on this quarter (bf16 matmuls) ----
            for j in range(KC):
                gwig = gwp.tile([P, 2 * D], f16, name=f"gwig_{q}_{j}", tag="gwig")
                nc.sync.dma_start(out=gwig,
                                  in_=gw_d[j].rearrange("p c f -> p (c f)"))
                gwfg = gwp.tile([P, 2 * D], f16, name=f"gwfg_{q}_{j}", tag="gwfg")
                nc.sync.dma_start(out=gwfg,
                                  in_=gw_d[j + KC].rearrange("p c f -> p (c f)"))

                ps_ig = psC_p.tile([P, QW], f32, name=f"psig_{q}_{j}", tag="ig")
                ps_fg = psC_p.tile([P, QW], f32, name=f"psfg_{q}_{j}", tag="fg")
                for c in range(GC):
                    rhs = (xTb[:, c * QW:(c + 1) * QW] if c < KC else
                           ffnTb[:, (c - KC) * QW:(c - KC + 1) * QW])
                    nc.tensor.matmul(ps_ig, gwig[:, c * P:(c + 1) * P], rhs,
                                     start=(c == 0), stop=(c == GC - 1))
                for c in range(GC):
                    rhs = (xTb[:, c * QW:(c + 1) * QW] if c < KC else
                           ffnTb[:, (c - KC) * QW:(c - KC + 1) * QW])
                    nc.tensor.matmul(ps_fg, gwfg[:, c * P:(c + 1) * P], rhs,
                                     start=(c == 0), stop=(c == GC - 1))

                sig_ig = sigp.tile([P, QW], f32, name=f"sigig_{q}_{j}", tag="ig")
                nc.scalar.activation(sig_ig, ps_ig, Act.Sigmoid,
                                     bias=gb_sb[:, j:j + 1])
                sig_fg = sigp.tile([P, QW], f32, name=f"sigfg_{q}_{j}", tag="fg")
                nc.scalar.activation(sig_fg, ps_fg, Act.Sigmoid,
                                     bias=gb_sb[:, j + KC:j + KC + 1])

                a = tmpp.tile([P, QW], f32r, name=f"a_{q}_{j}", tag="a")
                nc.vector.tensor_tensor(a, sig_ig, v(xT[:, j * QW:(j + 1) * QW]),
                                        op=Alu.mult)
                nc.vector.tensor_tensor(sig_fg, sig_fg,
                                        v(ffnT[:, j * QW:(j + 1) * QW]), op=Alu.mult)
                nc.vector.tensor_tensor(a, v(a), sig_fg, op=Alu.add)

                pt = trps_p.tile([P, 512], f32, name=f"pto{q}_{j}", tag="tr")
                for ti in range(QT):
                    nc.tensor.transpose(r(pt[:, ti * P:(ti + 1) * P]),
                                        a[:, ti * P:(ti + 1) * P], ident)
                po = piecep.tile([P, 512], f32, name=f"po{q}_{j}", tag="po")
                nc.vector.tensor_copy(po, pt)
                t0 = q * QW
                dram = out_d[t0:t0 + QW, j * P:(j + 1) * P]
                nc.sync.dma_start(out=dram.rearrange("(b p) f -> p b f", p=P),
                                  in_=wide3(po))

    nc.compile()
    _CACHE["nc"] = nc
    return nc


def _prep_maps(inputs, ln_g, ln_b, w1, b1, w2, b2, gw, gb):
    import ml_dtypes

    inputs = np.asarray(inputs, dtype=np.float32)
    ln_g = np.asarray(ln_g, dtype=np.float32)
    ln_b = np.asarray(ln_b, dtype=np.float32)
    w1 = np.asarray(w1, dtype=np.float32)
    b1 = np.asarray(b1, dtype=np.float32)
    w2 = np.asarray(w2, dtype=np.float32)
    b2 = np.asarray(b2, dtype=np.float32)
    gw = np.asarray(gw, dtype=np.float32)
    gb = np.asarray(gb, dtype=np.float32)

    w1f = (ln_g[:, None] * w1).astype(np.float32)
    b1f = (ln_b @ w1 + b1).astype(np.float32)

    base = {
        "w1": np.ascontiguousarray(w1f.reshape(KC, P, D)).astype(np.float16),
        "b1": np.ascontiguousarray(b1f.reshape(KC, P).T),
        "w2": np.ascontiguousarray(w2.reshape(KC, P, D)).astype(np.float16),
        "b2": np.ascontiguousarray(b2.reshape(KC, P).T),
        "gw": np.ascontiguousarray(
            gw.reshape(GC, P, GC, P).transpose(2, 1, 0, 3)).astype(np.float16),
        "gb": np.ascontiguousarray(gb.reshape(GC, P).T),
        "invsteps": np.ascontiguousarray(
            (1.0 / np.arange(1, L + 1, dtype=np.float32)).reshape(NT, P).T),
        "triu": np.triu(np.ones((P, P), np.float32)),
        "stril": np.tril(np.ones((P, P), np.float32), -1),
        "ident": np.eye(P, dtype=np.float32),
    }
    return [dict(base, x=np.ascontiguousarray(inputs[b])) for b in range(B)]


def _run(in_maps, trace=False):
    from concourse.bass_utils import run_bass_kernel_spmd
    nc = _build()
    return run_bass_kernel_spmd(nc, in_maps, list(range(B)), trace=trace)


def kernel(inputs, ln_g, ln_b, w1, b1, w2, b2, gw, gb):
    in_maps = _prep_maps(inputs, ln_g, ln_b, w1, b1, w2, b2, gw, gb)
    res = _run(in_maps).results
    out = np.stack([res[b]["out"] for b in range(B)])
    ffn = np.stack([res[b]["ffn"] for b in range(B)])
    return out, ffn


def kernel_traced(inputs, ln_g, ln_b, w1, b1, w2, b2, gw, gb):
    """Like kernel(), but also returns the BassKernelResults (with exec_time_ns)."""
    in_maps = _prep_maps(inputs, ln_g, ln_b, w1, b1, w2, b2, gw, gb)
    bkr = _run(in_maps, trace=True)
    res = bkr.results
    out = np.stack([res[b]["out"] for b in range(B)])
    ffn = np.stack([res[b]["ffn"] for b in range(B)])
    return (out, ffn), bkr
